# revision 22
# baseline (speedup 1.0000x reference)
"""Trainium2 Bass kernel for a GNN message-passing layer.

Reference computation (per batch b):
    m   = relu(h @ W1.T + b1)
    m   = relu(m @ W2.T + b2)
    msg = relu(A @ m)
    gx  = msg @ W_ih.T + b_ih ; gh = h @ W_hh.T + b_hh   (gates r,z,n)
    r = sig(gxr+ghr); z = sig(gxz+ghz); n = tanh(gxn + r*ghn)
    out = (1-z)*n + z*h

Sharding: pure data-parallel over B (B == n_cores == 8, one batch per
NeuronCore, no collectives). Host pre-transposes per-batch tensors into
feature-major layout so A streams through the PE in its natural layout.

Numerics/performance strategy:
  * The dominant A @ m2 matmul runs in float32r (fp32 data, TF32-like
    11-bit-mantissa rounding inside the PE, 4x the fp32 matmul rate).
  * A >= 0 (uniform) and m2 >= 0 (post-relu) imply msg >= 0, so the relu
    on msg is an identity. This makes msg exactly decomposable as
        msg = u (x) s  +  A @ (m2 - u),   s[n] = sum_m A[n, m]
    for any host-chosen u. With u ~= column means of m2 the residual is
    ~40x smaller than msg (~±10 vs ~400), so rounding the residual and
    the gate weights to f32r is numerically harmless, while rounding raw
    msg (~400) would corrupt the sigmoid/tanh pre-activations. The rank-1
    term v (x) s (v = W_ih @ u) is restored on the DVE. This turns ALL
    gate matmuls into fast f32r ones.
  * s is computed on the host from the f32r-rounded A so it matches what
    the PE accumulates; u and v are host fp64.
  * A is streamed as 16 x 1MB contiguous slabs (measured ~350GB/s).
    Host packs the slab content so that each quarter of the stream
    completes one 512-node chunk of msg, letting each chunk's GRU work
    overlap the next quarter's DMA (only the last chunk is a tail).
"""

import numpy as np

B, N, H = 8, 2048, 128
NCHUNK = 512
NCH = N // NCHUNK  # 4
KBLK = N // 128    # 16

_CACHE = {}


def _build_program():
    import concourse.bacc as bacc
    import concourse.tile as tile
    import concourse.mybir as mybir
    from concourse.alu_op_type import AluOpType

    f32 = mybir.dt.float32
    f32r = mybir.dt.float32r
    ACT = mybir.ActivationFunctionType

    nc = bacc.Bacc("TRN2", target_bir_lowering=False, debug=False, num_devices=B)

    # ---- DRAM I/O (per-core shard, host-prepacked) ----
    hT_d = nc.dram_tensor("hT", [H, N], f32, kind="ExternalInput").ap()
    # A2[q, s] = one contiguous [128, 2048] slab: 4 k-blocks (t=0..3, k=4s+t)
    # of A^T columns for node-chunk q.
    A2_d = nc.dram_tensor("A2", [NCH, KBLK // 4, H, N], f32r, kind="ExternalInput").ap()
    w1hl_d = nc.dram_tensor("W1hl", [H, 2 * H], f32r, kind="ExternalInput").ap()
    w2t_d = nc.dram_tensor("W2T", [H, H], f32, kind="ExternalInput").ap()
    wih_d = nc.dram_tensor("WihT", [H, 3 * H], f32r, kind="ExternalInput").ap()
    whh_d = nc.dram_tensor("WhhT", [H, 3 * H], f32r, kind="ExternalInput").ap()
    b1_d = nc.dram_tensor("b1c", [H, 1], f32, kind="ExternalInput").ap()
    b2b_d = nc.dram_tensor("b2b", [H, H], f32, kind="ExternalInput").ap()
    ub_d = nc.dram_tensor("ub", [H, H], f32, kind="ExternalInput").ap()
    brz_d = nc.dram_tensor("brz", [H, 2], f32, kind="ExternalInput").ap()
    bihn_d = nc.dram_tensor("bihn", [H, 1], f32, kind="ExternalInput").ap()
    bhhn_d = nc.dram_tensor("bhhn", [H, 1], f32, kind="ExternalInput").ap()
    v_d = nc.dram_tensor("vg", [H, 3], f32, kind="ExternalInput").ap()
    s_d = nc.dram_tensor("sbb", [H, N], f32, kind="ExternalInput").ap()
    out_d = nc.dram_tensor("outT", [H, N], f32, kind="ExternalOutput").ap()

    with tile.TileContext(nc) as tc:
        with (
            tc.tile_pool(name="consts", bufs=1) as cp,
            tc.tile_pool(name="big", bufs=1) as bp,
            tc.tile_pool(name="at", bufs=8) as ap_,
            tc.tile_pool(name="msgp", bufs=2) as mp,
            tc.tile_pool(name="tmp", bufs=2) as tp,
            tc.tile_pool(name="outp", bufs=2) as op_,
            tc.tile_pool(name="psum", bufs=1, space="PSUM") as pp,
        ):
            w1hl = cp.tile([H, 2 * H], f32r, tag="w1hl")
            w2t = cp.tile([H, H], f32, tag="w2t")
            wih = cp.tile([H, 3 * H], f32r, tag="wih")
            whh = cp.tile([H, 3 * H], f32r, tag="whh")
            b1 = cp.tile([H, 1], f32, tag="b1")
            b2b = cp.tile([H, H], f32, tag="b2b")
            ub = cp.tile([H, H], f32, tag="ub")
            brz = cp.tile([H, 2], f32, tag="brz")
            bihn = cp.tile([H, 1], f32, tag="bihn")
            bhhn = cp.tile([H, 1], f32, tag="bhhn")
            vg = cp.tile([H, 3], f32, tag="vg")
            sb = bp.tile([H, N], f32, tag="sb")
            hT = bp.tile([H, N], f32, tag="hT")
            hTr = bp.tile([H, N], f32r, tag="hTr")
            m1T = bp.tile([H, N], f32, tag="m1T")
            m2c = bp.tile([H, N], f32r, tag="m2c")  # (m2 - u), block k at cols 128k..

            # constants + hT on the ACT (scalar) HWDGE ring so the sync ring
            # streams A from t=0. hT in chunks; hTr = f32r copy for matmuls.
            nc.scalar.dma_start(w1hl[:], w1hl_d[:])
            for c in range(NCH):
                sl = slice(c * NCHUNK, (c + 1) * NCHUNK)
                nc.scalar.dma_start(hT[:, sl], hT_d[:, sl])
                nc.scalar.copy(hTr[:, sl], hT[:, sl])
            nc.scalar.dma_start(w2t[:], w2t_d[:])
            nc.scalar.dma_start(b1[:], b1_d[:])
            nc.scalar.dma_start(b2b[:], b2b_d[:])
            nc.scalar.dma_start(ub[:], ub_d[:])
            nc.scalar.dma_start(whh[:], whh_d[:])
            nc.scalar.dma_start(wih[:], wih_d[:])
            nc.scalar.dma_start(brz[:], brz_d[:])
            nc.scalar.dma_start(bihn[:], bihn_d[:])
            nc.scalar.dma_start(bhhn[:], bhhn_d[:])
            nc.scalar.dma_start(vg[:], v_d[:])
            nc.scalar.dma_start(sb[:], s_d[:])

            # ---- m1T = relu(W1 @ hT + b1): split-W1 f32r (exact W, h rounded) ----
            for c in range(NCH):
                sl = slice(c * NCHUNK, (c + 1) * NCHUNK)
                ps_m1 = pp.tile([H, NCHUNK], f32, tag="acc", bufs=5)
                nc.tensor.matmul(ps_m1[:], w1hl[:, 0:H], hTr[:, sl], start=True, stop=False)
                nc.tensor.matmul(ps_m1[:], w1hl[:, H:2 * H], hTr[:, sl], start=False, stop=True)
                nc.scalar.activation(m1T[:, sl], ps_m1[:], ACT.Relu, bias=b1[:, 0:1])

            # ---- m2c blocks: relu(m1T_k.T @ W2T + b2) - u  (node-major) ----
            for k in range(KBLK):
                kb = slice(k * H, (k + 1) * H)
                ps_m2 = pp.tile([H, H], f32, tag="acc", bufs=5)
                nc.tensor.matmul(ps_m2[:], m1T[:, kb], w2t[:], start=True, stop=True)
                m2pre = tp.tile([H, H], f32, tag="m2pre")
                nc.vector.tensor_add(m2pre[:], ps_m2[:], b2b[:])
                m2r = tp.tile([H, H], f32, tag="m2r")
                nc.scalar.activation(m2r[:], m2pre[:], ACT.Relu)
                nc.vector.tensor_sub(m2c[:, kb], m2r[:], ub[:])

            # ---- software-pipelined stream over quarters ----
            resids = [None] * NCH

            def emit_msg_quarter(q):
                ps_msg = pp.tile([H, NCHUNK], f32, tag="msg", bufs=3, name=f"psmsg{q}")
                for s_ in range(KBLK // 4):
                    at = ap_.tile([H, N], f32r, tag="at")
                    nc.sync.dma_start(at[:], A2_d[q, s_])
                    for t_ in range(4):
                        k = 4 * s_ + t_
                        nc.tensor.matmul(
                            ps_msg[:],
                            m2c[:, k * H:(k + 1) * H],
                            at[:, t_ * NCHUNK:(t_ + 1) * NCHUNK],
                            start=(k == 0), stop=(k == KBLK - 1),
                        )
                residT = mp.tile([H, NCHUNK], f32r, tag="residT", name=f"residT{q}")
                nc.scalar.copy(residT[:], ps_msg[:])
                resids[q] = residT

            def emit_gates(q):
                sl = slice(q * NCHUNK, (q + 1) * NCHUNK)
                residT = resids[q]

                # r gate: pre = vr*s + gh_r + gxR_r (+brz_r via sigmoid bias)
                ps_ghr = pp.tile([H, NCHUNK], f32, tag="acc", bufs=5)
                nc.tensor.matmul(ps_ghr[:], whh[:, 0:H], hTr[:, sl], start=True, stop=True)
                ghr = tp.tile([H, NCHUNK], f32, tag="ghr")
                nc.vector.scalar_tensor_tensor(
                    ghr[:], sb[:, sl], vg[:, 0:1], ps_ghr[:],
                    op0=AluOpType.mult, op1=AluOpType.add)
                ps_gxr = pp.tile([H, NCHUNK], f32, tag="acc", bufs=5)
                nc.tensor.matmul(ps_gxr[:], wih[:, 0:H], residT[:], start=True, stop=True)
                rpre = tp.tile([H, NCHUNK], f32, tag="rpre")
                nc.vector.tensor_add(rpre[:], ps_gxr[:], ghr[:])
                r = tp.tile([H, NCHUNK], f32, tag="r")
                nc.scalar.activation(r[:], rpre[:], ACT.Sigmoid, bias=brz[:, 0:1])

                # z gate
                ps_ghz = pp.tile([H, NCHUNK], f32, tag="acc", bufs=5)
                nc.tensor.matmul(ps_ghz[:], whh[:, H:2 * H], hTr[:, sl], start=True, stop=True)
                ghz = tp.tile([H, NCHUNK], f32, tag="ghz")
                nc.vector.scalar_tensor_tensor(
                    ghz[:], sb[:, sl], vg[:, 1:2], ps_ghz[:],
                    op0=AluOpType.mult, op1=AluOpType.add)
                ps_gxz = pp.tile([H, NCHUNK], f32, tag="acc", bufs=5)
                nc.tensor.matmul(ps_gxz[:], wih[:, H:2 * H], residT[:], start=True, stop=True)
                zpre = tp.tile([H, NCHUNK], f32, tag="zpre")
                nc.vector.tensor_add(zpre[:], ps_gxz[:], ghz[:])
                z = tp.tile([H, NCHUNK], f32, tag="z")
                nc.scalar.activation(z[:], zpre[:], ACT.Sigmoid, bias=brz[:, 1:2])

                # n gate: n = tanh((vn*s + gxR_n) + bihn + r*(gh_n + bhhn))
                ps_ghn = pp.tile([H, NCHUNK], f32, tag="acc", bufs=5)
                nc.tensor.matmul(ps_ghn[:], whh[:, 2 * H:3 * H], hTr[:, sl], start=True, stop=True)
                x = tp.tile([H, NCHUNK], f32, tag="x")
                nc.vector.scalar_tensor_tensor(
                    x[:], ps_ghn[:], bhhn[:, 0:1], r[:],
                    op0=AluOpType.add, op1=AluOpType.mult)   # x = (ghn+bhhn)*r
                ps_gxn = pp.tile([H, NCHUNK], f32, tag="acc", bufs=5)
                nc.tensor.matmul(ps_gxn[:], wih[:, 2 * H:3 * H], residT[:], start=True, stop=True)
                y = tp.tile([H, NCHUNK], f32, tag="y")
                nc.vector.scalar_tensor_tensor(
                    y[:], sb[:, sl], vg[:, 2:3], ps_gxn[:],
                    op0=AluOpType.mult, op1=AluOpType.add)   # y = vn*s + gxR_n
                npre = tp.tile([H, NCHUNK], f32, tag="npre")
                nc.vector.tensor_add(npre[:], x[:], y[:])
                nn = tp.tile([H, NCHUNK], f32, tag="nn")
                nc.scalar.activation(nn[:], npre[:], ACT.Tanh, bias=bihn[:, 0:1])

                # out = n + z * (h - n); early chunks on idle GPSIMD, last on DVE
                eng = nc.vector if q == NCH - 1 else nc.gpsimd
                d = tp.tile([H, NCHUNK], f32, tag="d")
                eng.tensor_sub(d[:], hT[:, sl], nn[:])
                e = tp.tile([H, NCHUNK], f32, tag="e")
                eng.tensor_mul(e[:], z[:], d[:])
                outc = op_.tile([H, NCHUNK], f32, tag="outc")
                eng.tensor_add(outc[:], nn[:], e[:])
                nc.scalar.dma_start(out_d[:, sl], outc[:])

            for q in range(NCH):
                emit_msg_quarter(q)
                if q >= 1:
                    emit_gates(q - 1)
            emit_gates(NCH - 1)

    nc.compile()
    return nc


def _get_program():
    if "nc" not in _CACHE:
        _CACHE["nc"] = _build_program()
    return _CACHE["nc"]


def _r32r(x):
    """Emulate the PE's f32r rounding: round-to-nearest at 11 mantissa bits."""
    u = np.asarray(x, np.float32).view(np.uint32)
    u2 = ((u.astype(np.uint64) + 0x800) & ~np.uint64(0xFFF)).astype(np.uint32)
    return u2.view(np.float32)


def _make_in_maps(h, A, W1, b1, W2, b2, W_ih, W_hh, b_ih, b_hh):
    f = np.float32
    h = np.asarray(h); A = np.asarray(A)
    W1 = np.asarray(W1); W2 = np.asarray(W2)
    W_ih = np.asarray(W_ih); W_hh = np.asarray(W_hh)
    b1 = np.asarray(b1); b2 = np.asarray(b2)
    b_ih = np.asarray(b_ih); b_hh = np.asarray(b_hh)

    W1T = np.ascontiguousarray(W1.T, dtype=f)
    w1hi = _r32r(W1T)
    w1lo = _r32r(W1T - w1hi)
    shared = {
        "W1hl": np.ascontiguousarray(np.concatenate([w1hi, w1lo], axis=1)),
        "W2T": np.ascontiguousarray(W2.T, dtype=f),
        "WihT": np.ascontiguousarray(W_ih.T, dtype=f),
        "WhhT": np.ascontiguousarray(W_hh.T, dtype=f),
        "b1c": np.ascontiguousarray(b1.reshape(H, 1), dtype=f),
        "b2b": np.ascontiguousarray(np.tile(b2.reshape(1, H), (H, 1)), dtype=f),
        "brz": np.ascontiguousarray(
            np.stack([(b_ih + b_hh)[0:H], (b_ih + b_hh)[H:2 * H]], axis=1), dtype=f),
        "bihn": np.ascontiguousarray(b_ih[2 * H:3 * H].reshape(H, 1), dtype=f),
        "bhhn": np.ascontiguousarray(b_hh[2 * H:3 * H].reshape(H, 1), dtype=f),
    }

    in_maps = []
    for bi in range(B):
        m = dict(shared)
        m["hT"] = np.ascontiguousarray(h[bi].T, dtype=f)
        AT = np.ascontiguousarray(A[bi].T, dtype=f)      # [2048 m, 2048 n]
        A2 = (AT.reshape(KBLK // 4, 4, H, NCH, NCHUNK)   # [s, t, p, q, j]
                .transpose(3, 0, 2, 1, 4)                # [q, s, p, t, j]
                .reshape(NCH, KBLK // 4, H, N))
        m["A2"] = np.ascontiguousarray(A2)

        # u = column means of m2 (host fp64 estimate; any u is algebraically
        # exact -- a good u just shrinks the streamed residual)
        h64 = h[bi].astype(np.float64)
        m1 = np.maximum(h64 @ W1.astype(np.float64).T + b1.astype(np.float64), 0)
        m2 = np.maximum(m1 @ W2.astype(np.float64).T + b2.astype(np.float64), 0)
        # u must be exactly f32r-representable: half of m2 is 0 (relu), so
        # m2c = -u there, and rounding that constant would be a systematic
        # error accumulating linearly over the K=2048 msg sum.
        u = _r32r(m2.mean(axis=0)).astype(np.float64)     # [H]
        v = W_ih.astype(np.float64) @ u                   # [3H]
        # s must match what the PE accumulates: row-sums of f32r-rounded A
        s = _r32r(A[bi]).astype(np.float64).sum(axis=1)   # [N]

        m["ub"] = np.ascontiguousarray(np.tile(u.astype(f).reshape(1, H), (H, 1)))
        m["vg"] = np.ascontiguousarray(
            np.stack([v[0:H], v[H:2 * H], v[2 * H:3 * H]], axis=1).astype(f))
        m["sbb"] = np.ascontiguousarray(np.tile(s.reshape(1, N).astype(f), (H, 1)))
        in_maps.append(m)
    return in_maps


def run(inputs, trace=False, trace_cores=None):
    """Build (cached), run on 8 cores, return (output, BassKernelResults)."""
    from concourse.bass_utils import run_bass_kernel_spmd

    nc = _get_program()
    in_maps = _make_in_maps(**inputs)
    res = run_bass_kernel_spmd(
        nc, in_maps, list(range(B)), trace=trace,
        trace_cores=trace_cores,
    )
    out = np.stack([res.results[b]["outT"].T for b in range(B)]).astype(np.float32)
    return out, res


def kernel(**inputs):
    out, _ = run(inputs, trace=False)
    return out


# revision 27
# speedup vs baseline: 1.0511x; 1.0511x over previous
"""Trainium2 Bass kernel for a GNN message-passing layer.

Reference computation (per batch b):
    m   = relu(h @ W1.T + b1)
    m   = relu(m @ W2.T + b2)
    msg = relu(A @ m)
    gx  = msg @ W_ih.T + b_ih ; gh = h @ W_hh.T + b_hh   (gates r,z,n)
    r = sig(gxr+ghr); z = sig(gxz+ghz); n = tanh(gxn + r*ghn)
    out = (1-z)*n + z*h

Sharding: pure data-parallel over B (B == n_cores == 8, one batch per
NeuronCore, no collectives). Host pre-transposes per-batch tensors into
feature-major layout so A streams through the PE in its natural layout.

Numerics/performance strategy:
  * The dominant A @ m2 matmul runs in float32r (fp32 data, TF32-like
    11-bit-mantissa rounding inside the PE, 4x the fp32 matmul rate).
  * A >= 0 (uniform) and m2 >= 0 (post-relu) imply msg >= 0, so the relu
    on msg is an identity. This makes msg exactly decomposable as
        msg = u (x) s  +  A @ (m2 - u),   s[n] = sum_m A[n, m]
    for any host-chosen u. With u ~= column means of m2 the residual is
    ~40x smaller than msg (~±10 vs ~400), so rounding the residual and
    the gate weights to f32r is numerically harmless, while rounding raw
    msg (~400) would corrupt the sigmoid/tanh pre-activations. The rank-1
    term v (x) s (v = W_ih @ u) is restored on the DVE. This turns ALL
    gate matmuls into fast f32r ones.
  * s is computed on the host from the f32r-rounded A so it matches what
    the PE accumulates; u and v are host fp64.
  * A is streamed as 16 x 1MB contiguous slabs (measured ~350GB/s).
    Host packs the slab content so that each quarter of the stream
    completes one 512-node chunk of msg, letting each chunk's GRU work
    overlap the next quarter's DMA (only the last chunk is a tail).
"""

import numpy as np

B, N, H = 8, 2048, 128
NCHUNK = 512
NCH = N // NCHUNK  # 4
KBLK = N // 128    # 16

_CACHE = {}


def _build_program():
    import concourse.bacc as bacc
    import concourse.tile as tile
    import concourse.mybir as mybir
    from concourse.alu_op_type import AluOpType

    f32 = mybir.dt.float32
    f32r = mybir.dt.float32r
    f16 = mybir.dt.float16
    ACT = mybir.ActivationFunctionType

    nc = bacc.Bacc("TRN2", target_bir_lowering=False, debug=False, num_devices=B)

    # ---- DRAM I/O (per-core shard, host-prepacked) ----
    hT_d = nc.dram_tensor("hT", [H, N], f32r, kind="ExternalInput").ap()
    # A2[q, g] = one contiguous [128, 4096] fp16 slab (1MB): 8 k-blocks
    # (t=0..7, k=8g+t) of A^T columns for node-chunk q.
    A2_d = nc.dram_tensor("A2", [NCH, KBLK // 8, H, 8 * NCHUNK], f16, kind="ExternalInput").ap()
    w1hl_d = nc.dram_tensor("W1hl", [H, 2 * H], f32r, kind="ExternalInput").ap()
    w2t_d = nc.dram_tensor("W2T", [H, H], f32, kind="ExternalInput").ap()
    wih_d = nc.dram_tensor("WihT", [H, 3 * H], f32r, kind="ExternalInput").ap()
    whh_d = nc.dram_tensor("WhhT", [H, 3 * H], f32r, kind="ExternalInput").ap()
    b1_d = nc.dram_tensor("b1c", [H, 1], f32, kind="ExternalInput").ap()
    b2b_d = nc.dram_tensor("b2b", [H, H], f32, kind="ExternalInput").ap()
    ub_d = nc.dram_tensor("ub", [H, H], f32, kind="ExternalInput").ap()
    brz_d = nc.dram_tensor("brz", [H, 2], f32, kind="ExternalInput").ap()
    bihn_d = nc.dram_tensor("bihn", [H, 1], f32, kind="ExternalInput").ap()
    bhhn_d = nc.dram_tensor("bhhn", [H, 1], f32, kind="ExternalInput").ap()
    v_d = nc.dram_tensor("vq", [4, 3 * H], f32r, kind="ExternalInput").ap()
    s_d = nc.dram_tensor("s4", [4, N], f32r, kind="ExternalInput").ap()
    out_d = nc.dram_tensor("outT", [H, N], f32, kind="ExternalOutput").ap()

    with tile.TileContext(nc) as tc:
        with (
            tc.tile_pool(name="consts", bufs=1) as cp,
            tc.tile_pool(name="big", bufs=1) as bp,
            tc.tile_pool(name="at", bufs=8) as ap_,
            tc.tile_pool(name="msgp", bufs=2) as mp,
            tc.tile_pool(name="tmp", bufs=2) as tp,
            tc.tile_pool(name="outp", bufs=2) as op_,
            tc.tile_pool(name="psum", bufs=1, space="PSUM") as pp,
        ):
            w1hl = cp.tile([H, 2 * H], f32r, tag="w1hl")
            w2t = cp.tile([H, H], f32, tag="w2t")
            wih = cp.tile([H, 3 * H], f32r, tag="wih")
            whh = cp.tile([H, 3 * H], f32r, tag="whh")
            b1 = cp.tile([H, 1], f32, tag="b1")
            b2b = cp.tile([H, H], f32, tag="b2b")
            ub = cp.tile([H, H], f32, tag="ub")
            brz = cp.tile([H, 2], f32, tag="brz")
            bihn = cp.tile([H, 1], f32, tag="bihn")
            bhhn = cp.tile([H, 1], f32, tag="bhhn")
            vqp = cp.tile([H, 3 * H], f32r, tag="vqp")
            s4p = bp.tile([H, N], f32r, tag="s4p")
            hTr = bp.tile([H, N], f32r, tag="hTr")
            m1T = bp.tile([H, N], f32, tag="m1T")
            m2c = bp.tile([H, N], f16, tag="m2c")  # (m2 - u), block k at cols 128k..

            # constants + hT on the ACT (scalar) HWDGE ring so the sync ring
            # streams A from t=0. hT in chunks; hTr = f32r copy for matmuls.
            nc.scalar.dma_start(w1hl[:], w1hl_d[:])
            for c in range(NCH):
                sl = slice(c * NCHUNK, (c + 1) * NCHUNK)
                nc.scalar.dma_start(hTr[:, sl], hT_d[:, sl])
            nc.scalar.dma_start(w2t[:], w2t_d[:])
            nc.scalar.dma_start(b1[:], b1_d[:])
            nc.scalar.dma_start(b2b[:], b2b_d[:])
            nc.scalar.dma_start(ub[:], ub_d[:])
            nc.scalar.dma_start(whh[:], whh_d[:])
            nc.scalar.dma_start(wih[:], wih_d[:])
            nc.scalar.dma_start(brz[:], brz_d[:])
            nc.scalar.dma_start(bihn[:], bihn_d[:])
            nc.scalar.dma_start(bhhn[:], bhhn_d[:])
            # zero-pad the 4-row v/s split factors to K=128 (PE needs full-K
            # stationary; zero rows contribute nothing)
            nc.vector.memset(vqp[:].bitcast(f32), 0.0)
            nc.gpsimd.memset(s4p[:].bitcast(f32), 0.0)
            nc.scalar.dma_start(vqp[0:4, :], v_d[:])
            nc.scalar.dma_start(s4p[0:4, :], s_d[:])

            # ---- m1T = relu(W1 @ hT + b1): split-W1 f32r (exact W, h rounded) ----
            for c in range(NCH):
                sl = slice(c * NCHUNK, (c + 1) * NCHUNK)
                ps_m1 = pp.tile([H, NCHUNK], f32, tag="acc", bufs=5)
                nc.tensor.matmul(ps_m1[:], w1hl[:, 0:H], hTr[:, sl], start=True, stop=False)
                nc.tensor.matmul(ps_m1[:], w1hl[:, H:2 * H], hTr[:, sl], start=False, stop=True)
                nc.scalar.activation(m1T[:, sl], ps_m1[:], ACT.Relu, bias=b1[:, 0:1])

            # ---- m2c blocks: relu(m1T_k.T @ W2T + b2) - u  (node-major) ----
            for k in range(KBLK):
                kb = slice(k * H, (k + 1) * H)
                ps_m2 = pp.tile([H, H], f32, tag="acc", bufs=5)
                nc.tensor.matmul(ps_m2[:], m1T[:, kb], w2t[:], start=True, stop=True)
                m2pre = tp.tile([H, H], f32, tag="m2pre")
                nc.vector.tensor_add(m2pre[:], ps_m2[:], b2b[:])
                m2r = tp.tile([H, H], f32, tag="m2r")
                nc.scalar.activation(m2r[:], m2pre[:], ACT.Relu)
                nc.vector.tensor_sub(m2c[:, kb], m2r[:], ub[:])

            # ---- software-pipelined stream over quarters ----
            resids = [None] * NCH

            def emit_msg_quarter(q):
                ps_msg = pp.tile([H, NCHUNK], f32, tag="msg", bufs=3, name=f"psmsg{q}")
                for g_ in range(KBLK // 8):
                    at = ap_.tile([H, 8 * NCHUNK], f16, tag="at")
                    nc.sync.dma_start(at[:], A2_d[q, g_])
                    for t_ in range(8):
                        k = 8 * g_ + t_
                        nc.tensor.matmul(
                            ps_msg[:],
                            m2c[:, k * H:(k + 1) * H],
                            at[:, t_ * NCHUNK:(t_ + 1) * NCHUNK],
                            start=(k == 0), stop=(k == KBLK - 1),
                        )
                residT = mp.tile([H, NCHUNK], f32r, tag="residT", name=f"residT{q}")
                nc.scalar.copy(residT[:], ps_msg[:])
                resids[q] = residT

            def emit_gates(q):
                sl = slice(q * NCHUNK, (q + 1) * NCHUNK)
                residT = resids[q]

                # r gate: ps_r = gh_r + v_r(x)s + gxR_r, sigmoid straight
                # from psum (brz_r via bias). v(x)s is an exact K=4 matmul:
                # rows [vhi;vhi;vlo;vlo] x [shi;slo;shi;slo].
                ps_r = pp.tile([H, NCHUNK], f32, tag="acc", bufs=5)
                nc.tensor.matmul(ps_r[:], whh[:, 0:H], hTr[:, sl], start=True, stop=False)
                nc.tensor.matmul(ps_r[:], vqp[:, 0:H], s4p[:, sl], start=False, stop=False)
                nc.tensor.matmul(ps_r[:], wih[:, 0:H], residT[:], start=False, stop=True)
                r = tp.tile([H, NCHUNK], f32, tag="r")
                nc.scalar.activation(r[:], ps_r[:], ACT.Sigmoid, bias=brz[:, 0:1])

                # z gate
                ps_z = pp.tile([H, NCHUNK], f32, tag="acc", bufs=5)
                nc.tensor.matmul(ps_z[:], whh[:, H:2 * H], hTr[:, sl], start=True, stop=False)
                nc.tensor.matmul(ps_z[:], vqp[:, H:2 * H], s4p[:, sl], start=False, stop=False)
                nc.tensor.matmul(ps_z[:], wih[:, H:2 * H], residT[:], start=False, stop=True)
                z = tp.tile([H, NCHUNK], f32, tag="z")
                nc.scalar.activation(z[:], ps_z[:], ACT.Sigmoid, bias=brz[:, 1:2])

                # n gate: n = tanh((vn(x)s + gxR_n) + bihn + r*(gh_n + bhhn))
                ps_ghn = pp.tile([H, NCHUNK], f32, tag="acc", bufs=5)
                nc.tensor.matmul(ps_ghn[:], whh[:, 2 * H:3 * H], hTr[:, sl], start=True, stop=True)
                x = tp.tile([H, NCHUNK], f32, tag="x")
                nc.vector.scalar_tensor_tensor(
                    x[:], ps_ghn[:], bhhn[:, 0:1], r[:],
                    op0=AluOpType.add, op1=AluOpType.mult)   # x = (ghn+bhhn)*r
                ps_gxn = pp.tile([H, NCHUNK], f32, tag="acc", bufs=5)
                nc.tensor.matmul(ps_gxn[:], vqp[:, 2 * H:3 * H], s4p[:, sl], start=True, stop=False)
                nc.tensor.matmul(ps_gxn[:], wih[:, 2 * H:3 * H], residT[:], start=False, stop=True)
                npre = tp.tile([H, NCHUNK], f32, tag="npre")
                nc.vector.tensor_add(npre[:], x[:], ps_gxn[:])
                nn = tp.tile([H, NCHUNK], f32, tag="nn")
                nc.scalar.activation(nn[:], npre[:], ACT.Tanh, bias=bihn[:, 0:1])

                # out = n + z * (h - n); early chunks on idle GPSIMD, last on DVE
                eng = nc.vector if q == NCH - 1 else nc.gpsimd
                d = tp.tile([H, NCHUNK], f32, tag="d")
                eng.tensor_sub(d[:], hTr[:, sl].bitcast(f32), nn[:])
                e = tp.tile([H, NCHUNK], f32, tag="e")
                eng.tensor_mul(e[:], z[:], d[:])
                outc = op_.tile([H, NCHUNK], f32, tag="outc")
                eng.tensor_add(outc[:], nn[:], e[:])
                nc.scalar.dma_start(out_d[:, sl], outc[:])

            for q in range(NCH):
                emit_msg_quarter(q)
                if q >= 1:
                    emit_gates(q - 1)
            emit_gates(NCH - 1)

    nc.compile()
    return nc


def _get_program():
    if "nc" not in _CACHE:
        _CACHE["nc"] = _build_program()
    return _CACHE["nc"]


def _r32r(x):
    """Emulate the PE's f32r rounding: round-to-nearest at 11 mantissa bits."""
    u = np.asarray(x, np.float32).view(np.uint32)
    u2 = ((u.astype(np.uint64) + 0x800) & ~np.uint64(0xFFF)).astype(np.uint32)
    return u2.view(np.float32)


def _make_in_maps(h, A, W1, b1, W2, b2, W_ih, W_hh, b_ih, b_hh):
    f = np.float32
    h = np.asarray(h); A = np.asarray(A)
    W1 = np.asarray(W1); W2 = np.asarray(W2)
    W_ih = np.asarray(W_ih); W_hh = np.asarray(W_hh)
    b1 = np.asarray(b1); b2 = np.asarray(b2)
    b_ih = np.asarray(b_ih); b_hh = np.asarray(b_hh)

    W1T = np.ascontiguousarray(W1.T, dtype=f)
    w1hi = _r32r(W1T)
    w1lo = _r32r(W1T - w1hi)
    shared = {
        "W1hl": np.ascontiguousarray(np.concatenate([w1hi, w1lo], axis=1)),
        "W2T": np.ascontiguousarray(W2.T, dtype=f),
        "WihT": np.ascontiguousarray(W_ih.T, dtype=f),
        "WhhT": np.ascontiguousarray(W_hh.T, dtype=f),
        "b1c": np.ascontiguousarray(b1.reshape(H, 1), dtype=f),
        "b2b": np.ascontiguousarray(np.tile(b2.reshape(1, H), (H, 1)), dtype=f),
        "brz": np.ascontiguousarray(
            np.stack([(b_ih + b_hh)[0:H], (b_ih + b_hh)[H:2 * H]], axis=1), dtype=f),
        "bihn": np.ascontiguousarray(b_ih[2 * H:3 * H].reshape(H, 1), dtype=f),
        "bhhn": np.ascontiguousarray(b_hh[2 * H:3 * H].reshape(H, 1), dtype=f),
    }

    in_maps = []
    for bi in range(B):
        m = dict(shared)
        m["hT"] = np.ascontiguousarray(h[bi].T, dtype=f)
        A16 = A[bi].astype(np.float16)
        AT = np.ascontiguousarray(A16.T)                  # [2048 m, 2048 n] fp16
        A2 = (AT.reshape(KBLK // 8, 8, H, NCH, NCHUNK)    # [g, t, p, q, j]
                .transpose(3, 0, 2, 1, 4)                 # [q, g, p, t, j]
                .reshape(NCH, KBLK // 8, H, 8 * NCHUNK))
        m["A2"] = np.ascontiguousarray(A2)

        # u = column means of m2 (host fp64 estimate; any u is algebraically
        # exact -- a good u just shrinks the streamed residual). u must be
        # exactly fp16-representable: half of m2 is 0 (relu), so m2c = -u
        # there, and rounding that constant would be a systematic error
        # accumulating linearly over the K=2048 msg sum.
        h64 = h[bi].astype(np.float64)
        m1 = np.maximum(h64 @ W1.astype(np.float64).T + b1.astype(np.float64), 0)
        m2 = np.maximum(m1 @ W2.astype(np.float64).T + b2.astype(np.float64), 0)
        u = m2.mean(axis=0).astype(np.float16).astype(np.float64)   # [H]
        v = W_ih.astype(np.float64) @ u                   # [3H]
        # s must match what the PE accumulates: row-sums of the fp16 A
        s = A16.astype(np.float64).sum(axis=1)            # [N]

        # split v and s into f32r hi+lo pairs; the K=4 matmul
        # [vhi;vhi;vlo;vlo].T @ [shi;slo;shi;slo] reconstructs v(x)s exactly
        v32 = v.astype(f); s32 = s.astype(f)
        vhi = _r32r(v32); vlo = _r32r(v32 - vhi)
        shi = _r32r(s32); slo = _r32r(s32 - shi)
        m["ub"] = np.ascontiguousarray(np.tile(u.astype(f).reshape(1, H), (H, 1)))
        m["vq"] = np.ascontiguousarray(np.stack([vhi, vhi, vlo, vlo], axis=0))
        m["s4"] = np.ascontiguousarray(np.stack([shi, slo, shi, slo], axis=0))
        in_maps.append(m)
    return in_maps


def run(inputs, trace=False, trace_cores=None):
    """Build (cached), run on 8 cores, return (output, BassKernelResults)."""
    from concourse.bass_utils import run_bass_kernel_spmd

    nc = _get_program()
    in_maps = _make_in_maps(**inputs)
    res = run_bass_kernel_spmd(
        nc, in_maps, list(range(B)), trace=trace,
        trace_cores=trace_cores,
    )
    out = np.stack([res.results[b]["outT"].T for b in range(B)]).astype(np.float32)
    return out, res


def kernel(**inputs):
    out, _ = run(inputs, trace=False)
    return out
